# revision 1
# baseline (speedup 1.0000x reference)
"""Trainium2 Bass kernel for the InteractPre co-attention module.

Math (reference):
    p  = relu(protein @ Wc.T + bc)           [L, 256]
    r  = relu(reactions @ W2.T + b2)         [Q, 64]
    k  = relu(p @ W1.T + b1)                 [L, 64]
    ra = r @ Wra.T + bra                     [Q, 64]
    pa = k @ Wpa.T + bpa                     [L, 64]
    A  = relu(ra[:,None,:] + pa[None,:,:]) @ Wa.T + ba   [Q, L, 64]
    r_gate = sigmoid(mean_l A);  p_gate = sigmoid(mean_q A)
    rxnfp = r*(1+r_gate); prot = max_l k*(1+p_gate)
    out = MLP(concat([rxnfp, prot]))         [Q]

Key optimization: A is never materialized.  Because the Wa matmul is linear,
    mean_l A[q] = (S_r[q]/L) @ Wa.T + ba   with  S_r[q] = sum_l relu(ra[q]+pa[l])
    mean_q A[l] = (S_p[l]/Q) @ Wa.T + ba   with  S_p[l] = sum_q relu(ra[q]+pa[l])
so only the two 64-channel sums are needed — O(Q*L*64) elementwise work, no
O(Q*L*64) HBM traffic (the naive version writes+reads ~1GB).

Sharding: L axis across the 8 cores (512 rows each).  The protein conv, the
pairwise stage and p_gate/prot are then fully local; only S_r (64x512 fp32)
needs an AllReduce, into which we piggyback the 8 per-core prot maxima.

Device layout: everything transposed (channels on partitions, tokens on the
free axis).  Pairwise stage: tile j holds tmp[c(64)x2, q(512)] =
relu(ra2 + pa2[:,j]) covering local rows l=j and l=j+256 in the two
partition halves.  Producers are split between ACT (activation w/ fused
free-axis accum -> S_p column) and DVE (tensor_scalar w/ accum); the
S_r accumulation runs on the otherwise-idle PE as a "fold" matmul
(lhsT = [I64;I64]) accumulating all 256 tiles into one PSUM bank.
"""

import os
import sys

import numpy as np

if "/opt/trn_rl_repo" not in sys.path:
    sys.path.insert(0, "/opt/trn_rl_repo")

Q = 512
L = 4096
NCORES = 8
L_LOC = L // NCORES          # 512 protein rows per core
NPAIR = L_LOC // 2           # 256 pairwise tiles per core
D = 64                       # co-attention channel count

# --- tunables -------------------------------------------------------------
# Tile j's producer goes to ACT iff (j % ACT_MOD) < ACT_NUM, else DVE.
ACT_MOD = int(os.environ.get("K_ACT_MOD", "7"))
ACT_NUM = int(os.environ.get("K_ACT_NUM", "3"))
PAIR_BF16 = os.environ.get("K_PAIR_BF16", "1") == "1"   # tmp tiles in bf16
TMP_BUFS = int(os.environ.get("K_TMP_BUFS", "8"))

_CACHE = {}


def _build():
    """Build + compile the SPMD Bass program (one program, 8 cores)."""
    import concourse.bass as bass
    import concourse.bacc as bacc
    import concourse.tile as tile
    from concourse import mybir

    f32 = mybir.dt.float32
    bf16 = mybir.dt.bfloat16
    tmp_dt = bf16 if PAIR_BF16 else f32
    AF = mybir.ActivationFunctionType
    ALU = mybir.AluOpType

    nc = bacc.Bacc("TRN2", target_bir_lowering=False, debug=False,
                   num_devices=NCORES)

    def din(name, shape, dt=f32):
        return nc.dram_tensor(name, list(shape), dt, kind="ExternalInput").ap()

    # ---- external inputs (host-prepped, transposed for lhsT use) ----
    protT = din("protT", [1024, L_LOC], bf16)        # per-core protein shard^T
    reactT = din("reactT", [256, Q])
    WcT = din("WcT", [1024, 256], bf16)
    W1T = din("W1T", [256, D])
    W2T = din("W2T", [256, D])
    WaT = din("WaT", [D, D])
    WpaT = din("WpaT", [D, D])
    WraT2 = din("WraT2", [D, 128])             # Wra.T duplicated columns
    Wf1aT = din("Wf1aT", [D, 256])             # Wf1[:, :64].T
    Wf1bT = din("Wf1bT", [D, 256])             # Wf1[:, 64:].T
    Wf2T = din("Wf2T", [256, 128])
    Wf3T = din("Wf3T", [128, 1])
    bc_d = din("bc", [256, 1])
    b1_d = din("b1", [D, 1])
    b2_d = din("b2", [D, 1])
    ba_d = din("ba", [D, 1])
    bpa_d = din("bpa", [D, 1])
    bra2_d = din("bra2", [128, 1])
    bf1_d = din("bf1", [256, 1])
    bf2_d = din("bf2", [128, 1])
    bf3_d = din("bf3", [1, 1])
    ifold_d = din("Ifold", [128, D], tmp_dt)   # [I64; I64]
    mask8_d = din("mask8", [D, NCORES])        # one-hot column = core id
    maskD_d = din("maskD", [128, NPAIR])       # 1.0 at DVE-produced cols

    out_d = nc.dram_tensor("out", [1, Q], f32, kind="ExternalOutput").ap()

    with tile.TileContext(nc) as tc:
        with (
            tc.tile_pool(name="const", bufs=1) as cp,
            tc.tile_pool(name="work", bufs=1) as wp,
            tc.tile_pool(name="tmp", bufs=TMP_BUFS) as tp,
            tc.tile_pool(name="psum", bufs=2, space="PSUM") as ps,
            tc.tile_pool(name="psum_sr", bufs=1, space="PSUM") as ps_sr,
            tc.tile_pool(name="dram", bufs=1, space="DRAM") as dp,
        ):
            dma = nc.sync.dma_start

            # ---------------- load constants ----------------
            def cload(src, shape, dt=f32, tag=None):
                t = cp.tile(list(shape), dt, tag=tag or src.tensor.name)
                dma(t[:], src)
                return t

            reactT_sb = [cload(reactT[i * 128:(i + 1) * 128, :], [128, Q],
                               tag=f"reactT{i}") for i in range(2)]
            W2T_sb = [cload(W2T[i * 128:(i + 1) * 128, :], [128, D],
                            tag=f"W2T{i}") for i in range(2)]
            WraT2_sb = cload(WraT2, [D, 128])
            bra2_sb = cload(bra2_d, [128, 1])
            b2_sb = cload(b2_d, [D, 1])
            WcT_sb = []
            protT_sb = []
            for i in range(8):
                protT_sb.append(cload(protT[i * 128:(i + 1) * 128, :],
                                      [128, L_LOC], bf16, tag=f"protT{i}"))
                WcT_sb.append(cload(WcT[i * 128:(i + 1) * 128, :],
                                    [128, 256], bf16, tag=f"WcT{i}"))
            W1T_sb = [cload(W1T[i * 128:(i + 1) * 128, :], [128, D],
                            tag=f"W1T{i}") for i in range(2)]
            WaT_sb = cload(WaT, [D, D])
            WpaT_sb = cload(WpaT, [D, D])
            Wf1aT_sb = cload(Wf1aT, [D, 256])
            Wf1bT_sb = cload(Wf1bT, [D, 256])
            Wf2T_sb = [cload(Wf2T[i * 128:(i + 1) * 128, :], [128, 128],
                             tag=f"Wf2T{i}") for i in range(2)]
            Wf3T_sb = cload(Wf3T, [128, 1])
            bc_sb = [cload(bc_d[i * 128:(i + 1) * 128, :], [128, 1],
                           tag=f"bc{i}") for i in range(2)]
            b1_sb = cload(b1_d, [D, 1])
            ba_sb = cload(ba_d, [D, 1])
            bpa_sb = cload(bpa_d, [D, 1])
            bf1_sb = [cload(bf1_d[i * 128:(i + 1) * 128, :], [128, 1],
                            tag=f"bf1{i}") for i in range(2)]
            bf2_sb = cload(bf2_d, [128, 1])
            bf3_sb = cload(bf3_d, [1, 1])
            ifold_sb = cload(ifold_d, [128, D], tmp_dt)
            mask8_sb = cload(mask8_d, [D, NCORES])
            maskD_sb = cload(maskD_d, [128, NPAIR])

            # ---------------- reaction side (replicated) ----------------
            psum_r = ps.tile([D, Q], f32, tag="ps_a")
            nc.tensor.matmul(psum_r[:], W2T_sb[0][:], reactT_sb[0][:],
                             start=True, stop=False)
            nc.tensor.matmul(psum_r[:], W2T_sb[1][:], reactT_sb[1][:],
                             start=False, stop=True)
            r_sb = wp.tile([D, Q], f32)
            nc.scalar.activation(r_sb[:], psum_r[:], AF.Relu, bias=b2_sb[:])

            # ra duplicated into both partition halves via doubled lhsT
            psum_ra2 = ps.tile([128, Q], f32, tag="ps_b")
            nc.tensor.matmul(psum_ra2[:], WraT2_sb[:], r_sb[:],
                             start=True, stop=True)
            ra2_sb = wp.tile([128, Q], tmp_dt)
            nc.scalar.activation(ra2_sb[:], psum_ra2[:], AF.Identity,
                                 bias=bra2_sb[:])

            # ---------------- protein side (sharded) ----------------
            p_sb = []
            for m in range(2):
                psum_p = ps.tile([128, L_LOC], f32, tag="ps_a")
                for i in range(8):
                    nc.tensor.matmul(
                        psum_p[:],
                        WcT_sb[i][:, m * 128:(m + 1) * 128],
                        protT_sb[i][:],
                        start=(i == 0), stop=(i == 7))
                pt = wp.tile([128, L_LOC], f32, tag=f"p{m}")
                nc.scalar.activation(pt[:], psum_p[:], AF.Relu,
                                     bias=bc_sb[m][:])
                p_sb.append(pt)

            psum_k = ps.tile([D, L_LOC], f32, tag="ps_a")
            nc.tensor.matmul(psum_k[:], W1T_sb[0][:], p_sb[0][:],
                             start=True, stop=False)
            nc.tensor.matmul(psum_k[:], W1T_sb[1][:], p_sb[1][:],
                             start=False, stop=True)
            k_sb = wp.tile([D, L_LOC], f32)
            nc.scalar.activation(k_sb[:], psum_k[:], AF.Relu, bias=b1_sb[:])

            psum_pa = ps.tile([D, L_LOC], f32, tag="ps_a")
            nc.tensor.matmul(psum_pa[:], WpaT_sb[:], k_sb[:],
                             start=True, stop=True)
            pa_sb = wp.tile([D, L_LOC], f32)
            nc.scalar.activation(pa_sb[:], psum_pa[:], AF.Identity,
                                 bias=bpa_sb[:])

            # pa2: split halves of local l onto the two partition halves
            pa2_sb = wp.tile([128, NPAIR], f32)
            dma(pa2_sb[0:D, :], pa_sb[:, 0:NPAIR])
            dma(pa2_sb[D:128, :], pa_sb[:, NPAIR:L_LOC])

            # ---------------- pairwise stage ----------------
            # DVE tiles use the shift trick: relu(ra+pa) = max(ra,-pa) + pa,
            # so one tensor_scalar yields the (shifted) tile AND its q-sum.
            # Shifts are repaired linearly: S_p col += (Q-1)*pa (masked);
            # the fold's S_r picks up -sum_{l in DVE} pa, repaired through
            # the AllReduce payload into the r_gate sigmoid bias.
            npa2_sb = wp.tile([128, NPAIR], f32)
            nc.vector.tensor_scalar_mul(npa2_sb[:], pa2_sb[:], -1.0)
            SpA = wp.tile([128, NPAIR], f32)    # S_p cols from ACT route
            SpD = wp.tile([128, NPAIR], f32)    # S_p cols from DVE route
            nc.gpsimd.memset(SpA[:], 0.0)
            nc.gpsimd.memset(SpD[:], 0.0)

            psum_Sr_a = ps_sr.tile([D, Q], f32, tag="sra")
            psum_Sr_b = ps_sr.tile([D, Q], f32, tag="srb")
            for j in range(NPAIR):
                tmp = tp.tile([128, Q], tmp_dt, tag="tmp")
                col = pa2_sb[:, j:j + 1]
                if (j % ACT_MOD) < ACT_NUM:
                    nc.scalar.activation(tmp[:], ra2_sb[:], AF.Relu,
                                         bias=col,
                                         accum_out=SpA[:, j:j + 1])
                else:
                    nc.vector.tensor_scalar(
                        tmp[:], ra2_sb[:], npa2_sb[:, j:j + 1], col,
                        ALU.max, ALU.add,
                        accum_out=SpD[:, j:j + 1])
                bank = psum_Sr_a if (j % 2 == 0) else psum_Sr_b
                nc.tensor.matmul(bank[:], ifold_sb[:], tmp[:],
                                 start=(j < 2), stop=(j >= NPAIR - 2))

            # ---------------- p_gate / prot (local) ----------------
            rad_sb = wp.tile([128, NPAIR], f32)
            nc.vector.tensor_tensor(rad_sb[:], pa2_sb[:], maskD_sb[:],
                                    op=ALU.mult)
            Sp2r = wp.tile([128, NPAIR], f32)
            nc.vector.tensor_tensor(Sp2r[:], SpA[:], SpD[:], op=ALU.add)
            Sp2 = wp.tile([128, NPAIR], f32)
            nc.vector.scalar_tensor_tensor(Sp2[:], rad_sb[:], float(Q - 1),
                                           Sp2r[:], op0=ALU.mult,
                                           op1=ALU.add)
            sumpa2 = wp.tile([128, 1], f32)
            nc.vector.reduce_sum(sumpa2[:], rad_sb[:],
                                 axis=mybir.AxisListType.X)
            Sp_hi = wp.tile([D, NPAIR], f32)
            dma(Sp_hi[:], Sp2[D:128, :])

            psum_pg = ps.tile([D, L_LOC], f32, tag="ps_a")
            nc.tensor.matmul(psum_pg[:, 0:NPAIR], WaT_sb[:], Sp2[0:D, :],
                             start=True, stop=True)
            nc.tensor.matmul(psum_pg[:, NPAIR:L_LOC], WaT_sb[:], Sp_hi[:],
                             start=True, stop=True)
            pgate_sb = wp.tile([D, L_LOC], f32)
            nc.scalar.activation(pgate_sb[:], psum_pg[:], AF.Sigmoid,
                                 bias=ba_sb[:], scale=1.0 / Q)

            g_sb = wp.tile([D, L_LOC], f32)
            nc.vector.scalar_tensor_tensor(g_sb[:], pgate_sb[:], 1.0,
                                           k_sb[:], op0=ALU.add,
                                           op1=ALU.mult)
            prot_sb = wp.tile([D, 1], f32)
            nc.vector.reduce_max(prot_sb[:], g_sb[:],
                                 axis=mybir.AxisListType.X)
            protcols_sb = wp.tile([D, NCORES], bf16)
            nc.vector.tensor_scalar_mul(protcols_sb[:], mask8_sb[:],
                                        prot_sb[:])

            Sr_half = wp.tile([D, Q], f32)
            nc.scalar.activation(Sr_half[:], psum_Sr_a[:], AF.Copy)
            Sr_sb = wp.tile([D, Q], bf16)
            nc.vector.tensor_tensor(Sr_sb[:], Sr_half[:], psum_Sr_b[:],
                                    op=ALU.add)

            # ---------------- collective ----------------
            NEX = NCORES + 2
            cc_in = dp.tile([D, Q + NEX], bf16)
            cc_out = dp.tile([D, Q + NEX], bf16, addr_space="Shared")
            dma(cc_in[:, 0:Q], Sr_sb[:])
            dma(cc_in[:, Q:Q + NCORES], protcols_sb[:])
            sumpa_bf = wp.tile([128, 1], bf16)
            nc.vector.tensor_scalar_mul(sumpa_bf[:], sumpa2[:], 1.0)
            dma(cc_in[:, Q + NCORES:Q + NCORES + 1], sumpa_bf[0:D, :])
            dma(cc_in[:, Q + NCORES + 1:Q + NEX], sumpa_bf[D:128, :])
            nc.gpsimd.collective_compute(
                "AllReduce", ALU.add,
                replica_groups=[list(range(NCORES))],
                ins=[cc_in[:].opt()],
                outs=[cc_out[:].opt()],
            )
            Srt_bf = wp.tile([D, Q], bf16)
            dma(Srt_bf[:], cc_out[:, 0:Q])
            Srt_sb = wp.tile([D, Q], f32)
            nc.scalar.activation(Srt_sb[:], Srt_bf[:], AF.Copy)
            prota_sb = wp.tile([D, NCORES], bf16)
            dma(prota_sb[:], cc_out[:, Q:Q + NCORES])
            spg_a = wp.tile([D, 1], bf16)
            spg_b = wp.tile([D, 1], bf16)
            dma(spg_a[:], cc_out[:, Q + NCORES:Q + NCORES + 1])
            dma(spg_b[:], cc_out[:, Q + NCORES + 1:Q + NEX])

            # ---------------- r_gate / head (replicated) ----------------
            protg_sb = wp.tile([D, 1], f32)
            nc.vector.reduce_max(protg_sb[:], prota_sb[:],
                                 axis=mybir.AxisListType.X)

            # r_gate bias: ba + Wa @ sumpa_glob / L  (un-shifts the fold)
            sumpa_g = wp.tile([D, 1], f32)
            nc.vector.tensor_tensor(sumpa_g[:], spg_a[:], spg_b[:],
                                    op=ALU.add)
            psum_bb = ps.tile([D, 1], f32, tag="ps_t")
            nc.tensor.matmul(psum_bb[:], WaT_sb[:], sumpa_g[:],
                             start=True, stop=True)
            babar_sb = wp.tile([D, 1], f32)
            nc.scalar.activation(babar_sb[:], psum_bb[:], AF.Identity,
                                 bias=ba_sb[:], scale=1.0 / L)

            psum_rg = ps.tile([D, Q], f32, tag="ps_a")
            nc.tensor.matmul(psum_rg[:], WaT_sb[:], Srt_sb[:],
                             start=True, stop=True)
            rgate_sb = wp.tile([D, Q], f32)
            nc.scalar.activation(rgate_sb[:], psum_rg[:], AF.Sigmoid,
                                 bias=babar_sb[:], scale=1.0 / L)
            rx_sb = wp.tile([D, Q], f32)
            nc.vector.scalar_tensor_tensor(rx_sb[:], rgate_sb[:], 1.0,
                                           r_sb[:], op0=ALU.add,
                                           op1=ALU.mult)

            h1_sb = []
            for m in range(2):
                psum_t = ps.tile([128, 1], f32, tag="ps_t")
                nc.tensor.matmul(psum_t[:],
                                 Wf1bT_sb[:, m * 128:(m + 1) * 128],
                                 protg_sb[:], start=True, stop=True)
                fold_sb = wp.tile([128, 1], f32, tag=f"fold{m}")
                nc.scalar.activation(fold_sb[:], psum_t[:], AF.Identity,
                                     bias=bf1_sb[m][:])
                psum_h1 = ps.tile([128, Q], f32, tag="ps_b")
                nc.tensor.matmul(psum_h1[:],
                                 Wf1aT_sb[:, m * 128:(m + 1) * 128],
                                 rx_sb[:], start=True, stop=True)
                h1l = wp.tile([128, Q], f32, tag=f"h1l{m}")
                nc.scalar.activation(h1l[:], psum_h1[:], AF.Identity,
                                     bias=fold_sb[:])
                h1 = wp.tile([128, Q], f32, tag=f"h1{m}")
                # leaky_relu(x) = max(0.01*x, x)
                nc.vector.scalar_tensor_tensor(h1[:], h1l[:], 0.01, h1l[:],
                                               op0=ALU.mult, op1=ALU.max)
                h1_sb.append(h1)

            psum_h2 = ps.tile([128, Q], f32, tag="ps_a")
            nc.tensor.matmul(psum_h2[:], Wf2T_sb[0][:], h1_sb[0][:],
                             start=True, stop=False)
            nc.tensor.matmul(psum_h2[:], Wf2T_sb[1][:], h1_sb[1][:],
                             start=False, stop=True)
            h2l_sb = wp.tile([128, Q], f32)
            nc.scalar.activation(h2l_sb[:], psum_h2[:], AF.Identity,
                                 bias=bf2_sb[:])
            h2_sb = wp.tile([128, Q], f32)
            nc.vector.scalar_tensor_tensor(h2_sb[:], h2l_sb[:], 0.01,
                                           h2l_sb[:], op0=ALU.mult,
                                           op1=ALU.max)

            psum_o = ps.tile([1, Q], f32, tag="ps_t")
            nc.tensor.matmul(psum_o[:], Wf3T_sb[:], h2_sb[:],
                             start=True, stop=True)
            out_sb = wp.tile([1, Q], f32)
            nc.scalar.activation(out_sb[:], psum_o[:], AF.Identity,
                                 bias=bf3_sb[:])
            dma(out_d, out_sb[:])

    nc.compile()
    return nc


def _get_nc():
    key = (ACT_MOD, ACT_NUM, PAIR_BF16, TMP_BUFS)
    if key not in _CACHE:
        _CACHE[key] = _build()
    return _CACHE[key]


def _prep_in_maps(inputs):
    from concourse import mybir
    bf16_np = mybir.dt.np(mybir.dt.bfloat16)
    tmp_np = bf16_np if PAIR_BF16 else np.float32

    f = lambda x: np.ascontiguousarray(np.asarray(x), dtype=np.float32)
    protein = f(inputs["protein"])[0]          # [L, 1024]
    reactions = f(inputs["reactions"])[0]      # [Q, 256]
    Wc, bc = f(inputs["Wc"]), f(inputs["bc"])
    W1, b1 = f(inputs["W1"]), f(inputs["b1"])
    W2, b2 = f(inputs["W2"]), f(inputs["b2"])
    Wa, ba = f(inputs["Wa"]), f(inputs["ba"])
    Wpa, bpa = f(inputs["Wpa"]), f(inputs["bpa"])
    Wra, bra = f(inputs["Wra"]), f(inputs["bra"])
    Wf1, bf1 = f(inputs["Wf1"]), f(inputs["bf1"])
    Wf2, bf2 = f(inputs["Wf2"]), f(inputs["bf2"])
    Wf3, bf3 = f(inputs["Wf3"]), f(inputs["bf3"])

    c = np.ascontiguousarray
    common = {
        "reactT": c(reactions.T),
        "WcT": c(Wc.T).astype(bf16_np),
        "W1T": c(W1.T),
        "W2T": c(W2.T),
        "WaT": c(Wa.T),
        "WpaT": c(Wpa.T),
        "WraT2": c(np.concatenate([Wra.T, Wra.T], axis=1)),
        "Wf1aT": c(Wf1[:, :D].T),
        "Wf1bT": c(Wf1[:, D:].T),
        "Wf2T": c(Wf2.T),
        "Wf3T": c(Wf3.T),
        "bc": bc.reshape(-1, 1),
        "b1": b1.reshape(-1, 1),
        "b2": b2.reshape(-1, 1),
        "ba": ba.reshape(-1, 1),
        "bpa": bpa.reshape(-1, 1),
        "bra2": np.tile(bra.reshape(-1, 1), (2, 1)),
        "bf1": bf1.reshape(-1, 1),
        "bf2": bf2.reshape(-1, 1),
        "bf3": bf3.reshape(-1, 1),
        "Ifold": np.concatenate([np.eye(D), np.eye(D)],
                                axis=0).astype(tmp_np),
    }
    is_dve = np.array([(j % ACT_MOD) >= ACT_NUM for j in range(NPAIR)],
                      np.float32)
    common["maskD"] = np.tile(is_dve, (128, 1)).astype(np.float32)
    in_maps = []
    for d in range(NCORES):
        shard = c(protein[d * L_LOC:(d + 1) * L_LOC, :].T).astype(bf16_np)
        mask8 = np.zeros((D, NCORES), np.float32)
        mask8[:, d] = 1.0
        in_maps.append({**common, "protT": shard, "mask8": mask8})
    return in_maps


def run(inputs, trace=False, **kw):
    from concourse import bass_utils
    nc = _get_nc()
    in_maps = _prep_in_maps(inputs)
    res = bass_utils.run_bass_kernel_spmd(
        nc, in_maps, core_ids=list(range(NCORES)), trace=trace, **kw)
    return res


def kernel(**inputs):
    res = run(inputs)
    return np.asarray(res.results[0]["out"], np.float32).reshape(-1)



# revision 4
# speedup vs baseline: 1.9862x; 1.9862x over previous
"""Trainium2 Bass kernel for the InteractPre co-attention module.

Math (reference):
    p  = relu(protein @ Wc.T + bc)           [L, 256]
    r  = relu(reactions @ W2.T + b2)         [Q, 64]
    k  = relu(p @ W1.T + b1)                 [L, 64]
    ra = r @ Wra.T + bra                     [Q, 64]
    pa = k @ Wpa.T + bpa                     [L, 64]
    A  = relu(ra[:,None,:] + pa[None,:,:]) @ Wa.T + ba   [Q, L, 64]
    r_gate = sigmoid(mean_l A);  p_gate = sigmoid(mean_q A)
    rxnfp = r*(1+r_gate); prot = max_l k*(1+p_gate)
    out = MLP(concat([rxnfp, prot]))         [Q]

Key optimization (vs the O(Q*L*64) elementwise pairwise stage): per channel c,
    S_r[q,c] = sum_l relu(ra[q,c] + pa[l,c]) = f_c(ra[q,c])
is a 1-D convex piecewise-linear function of ra[q,c] alone (and symmetrically
S_p[l,c] = g_c(pa[l,c])).  We tabulate f_c / g_c at B=16 shared symmetric
knots t_b (cost B*L*64, sharded) and evaluate via the relu-basis expansion
    fhat(x) = F_0 + s_0 (x - t_0) + sum_b w_b relu(x - t_b),
where w_b are second differences of the table.  Because the knot grid is
symmetric (-t_b = t_{B-1-b}), the *table-build* tiles relu(x + t_b) double as
the *evaluation* basis tiles relu(x - t_{B-1-b}), so 16 [128,512] elementwise
instructions per core replace the baseline's 256.  The weighted basis sums run
as diag(w) matmuls on the otherwise idle PE.

Sharding: L across 8 cores.  Only the F table (f's per-shard table, [64,16])
plus the per-core prot maxima cross cores: a single tiny [64,24] f32 AllReduce.
All dtypes fp16 (not bf16) for 8x less rounding noise; all matmuls fp16
single-pass.
"""

import os
import sys

import numpy as np

if "/opt/trn_rl_repo" not in sys.path:
    sys.path.insert(0, "/opt/trn_rl_repo")

Q = 512
L = 4096
NCORES = 8
L_LOC = L // NCORES          # 512 protein rows per core
D = 64                       # co-attention channel count
B = 16                       # knots (must be 16 for the pair layout below)
NB2 = B // 2                 # 8 knot-pair instructions
KR = float(os.environ.get("K_KR", "1.3"))   # knot range [-KR, KR]

# build knots on host: exactly symmetric so -t[b] == t[B-1-b] bitwise
_t = np.linspace(-KR, KR, B, dtype=np.float64)
_t = ((_t - _t[::-1]) / 2).astype(np.float32)
KH = float(_t[1] - _t[0])

# route: which knot-pair instructions go to ACT (others DVE)
ACT_ROUTE = tuple(int(c) for c in os.environ.get("K_ACT_ROUTE", "0101010101"))

_CACHE = {}


def _build():
    import concourse.bass as bass
    import concourse.bacc as bacc
    import concourse.tile as tile
    from concourse import mybir

    f32 = mybir.dt.float32
    f16 = mybir.dt.float16
    AF = mybir.ActivationFunctionType
    ALU = mybir.AluOpType
    AX = mybir.AxisListType

    nc = bacc.Bacc("TRN2", target_bir_lowering=False, debug=False,
                   num_devices=NCORES)

    def din(name, shape, dt=f32):
        return nc.dram_tensor(name, list(shape), dt, kind="ExternalInput").ap()

    # ---- external inputs (host-packed blobs) ----
    # cst32 [128, 27]: f32 biases + knot columns
    #  col 0: b2 (rows 0:64)      col 1: bra2 [128]
    #  col 2,3: bc halves         col 4: b1 (0:64)
    #  col 5: bpa2 [128]          col 6: ba (0:64)
    #  col 7,8: bf1 halves        col 9: bf2 [128]
    #  col 10: bf3 (row 0)
    #  cols 11:19  tcol_j  = [t_j ; t_{j+8}]
    #  cols 19:27  ntcol_j = [-t_j ; -t_{j+8}]
    cst32_d = din("cst32", [128, 27], f32)
    # cstA [128, 1280] fp16: reactT (0:1024), W2T 2 chunks (1024:1152),
    #   WraT2 (1152:1280, rows 0:64)
    cstA_d = din("cstA", [128, 1280], f16)
    # cstW [128, 2048] fp16: WcT 8 chunks of [128,256]
    cstW_d = din("cstW", [128, 2048], f16)
    # cstB [128, 1281] fp16: W1T 2 chunks (0:128), WpaT2 (128:256, rows 0:64),
    #   WaT (256:320, rows 0:64), Wf1aT (320:576, rows 0:64),
    #   Wf1bT (576:832, rows 0:64), Wf2T 2 chunks (832:1088),
    #   Wf3T (1088:1089), I2h (1089:1153), Itoph (1153:1217)
    cstB_d = din("cstB", [128, 1217], f16)
    protT_d = din("protT", [128, 4096], f16)      # per-core shard, 8 chunks
    mask8_d = din("mask8", [64, NCORES], f32)     # one-hot col = core id

    out_d = nc.dram_tensor("out", [1, Q], f32, kind="ExternalOutput").ap()

    with tile.TileContext(nc) as tc:
        with (
            tc.tile_pool(name="cp", bufs=1) as cp,
            tc.tile_pool(name="wp", bufs=1) as wp,
            tc.tile_pool(name="ps", bufs=1, space="PSUM") as ps,
            tc.tile_pool(name="dp", bufs=1, space="DRAM") as dp,
        ):
            dmas = nc.sync.dma_start       # sync-queue DMA
            dmaa = nc.scalar.dma_start     # act-queue DMA

            # ---------- ACT table preload: dummy sigmoid ----------
            dums = wp.tile([1, 1], f32)
            nc.gpsimd.memset(dums[:], 0.0)
            dumo = wp.tile([1, 1], f32)
            nc.scalar.activation(dumo[:], dums[:], AF.Sigmoid)

            # ---------- constant loads ----------
            cst32 = cp.tile([128, 27], f32)
            dmas(cst32[:], cst32_d)
            cstA = cp.tile([128, 1280], f16)
            dmas(cstA[:], cstA_d)
            cstW = cp.tile([128, 2048], f16)
            dmas(cstW[:], cstW_d)
            protT = cp.tile([128, 4096], f16)
            dmaa(protT[:, 0:2048], protT_d[:, 0:2048])
            dmaa(protT[:, 2048:4096], protT_d[:, 2048:4096])
            cstB = cp.tile([128, 1217], f16)
            dmas(cstB[:], cstB_d)
            mask8 = cp.tile([64, NCORES], f32)
            dmaa(mask8[:], mask8_d)

            b2c = cst32[0:64, 0:1]
            bra2c = cst32[:, 1:2]
            bcc = [cst32[:, 2:3], cst32[:, 3:4]]
            b1c = cst32[0:64, 4:5]
            bpa2c = cst32[:, 5:6]
            bac = cst32[0:64, 6:7]
            bf1c = [cst32[:, 7:8], cst32[:, 8:9]]
            bf2c = cst32[:, 9:10]
            bf3c = cst32[0:1, 10:11]
            tcol = [cst32[:, 11 + j:12 + j] for j in range(NB2)]
            ntcol = [cst32[:, 19 + j:20 + j] for j in range(NB2)]

            reactT = [cstA[:, 0:512], cstA[:, 512:1024]]
            W2T = [cstA[:, 1024:1088], cstA[:, 1088:1152]]
            WraT2 = cstA[0:64, 1152:1280]
            W1T = [cstB[:, 0:64], cstB[:, 64:128]]
            WpaT2 = cstB[0:64, 128:256]
            WaT = cstB[0:64, 256:320]
            Wf1aT = cstB[0:64, 320:576]
            Wf1bT = cstB[0:64, 576:832]
            Wf2T = [cstB[:, 832:960], cstB[:, 960:1088]]
            Wf3T = cstB[:, 1088:1089]
            I2h = cstB[:, 1089:1153]
            Itoph = cstB[:, 1153:1217]

            # ---------- reaction side ----------
            psum_r = ps.tile([D, Q], f32, tag="pa")
            nc.tensor.matmul(psum_r[:], W2T[0][:], reactT[0][:],
                             start=True, stop=False)
            nc.tensor.matmul(psum_r[:], W2T[1][:], reactT[1][:],
                             start=False, stop=True)
            r16 = wp.tile([D, Q], f16)
            nc.scalar.activation(r16[:], psum_r[:], AF.Relu, bias=b2c)

            psum_ra2 = ps.tile([128, Q], f32, tag="pb")
            nc.tensor.matmul(psum_ra2[:], WraT2, r16[:], start=True, stop=True)
            ra2 = wp.tile([128, Q], f16)
            nc.scalar.activation(ra2[:], psum_ra2[:], AF.Identity, bias=bra2c)

            # ---------- G table build (from ra2) + S_r eval tiles ----------
            # knot-pair j: top half knot t_j, bottom half t_{j+8}
            # tile TR_j = relu(ra + t)  (doubles as S_r basis at tau=-t)
            G2 = wp.tile([128, NB2], f32)
            TR = []
            for j in range(NB2):
                tr = wp.tile([128, Q], f16, name=f"TR{j}", tag=f"TR{j}")
                if ACT_ROUTE[j]:
                    nc.scalar.activation(tr[:], ra2[:], AF.Relu,
                                         bias=tcol[j],
                                         accum_out=G2[:, j:j + 1])
                else:
                    nc.vector.tensor_scalar(tr[:], ra2[:], ntcol[j], tcol[j],
                                            ALU.max, ALU.add)
                    nc.vector.reduce_sum(G2[:, j:j + 1], tr[:], axis=AX.X)
                TR.append(tr)

            # ---------- protein side (sharded) ----------
            p16 = []
            for m in range(2):
                psum_p = ps.tile([128, L_LOC], f32, tag=f"pc{m}")
                for i in range(8):
                    nc.tensor.matmul(
                        psum_p[:],
                        cstW[:, i * 256 + m * 128: i * 256 + (m + 1) * 128],
                        protT[:, i * 512:(i + 1) * 512],
                        start=(i == 0), stop=(i == 7))
                pt = wp.tile([128, L_LOC], f16, name=f"p16_{m}")
                nc.scalar.activation(pt[:], psum_p[:], AF.Relu, bias=bcc[m])
                p16.append(pt)

            psum_k = ps.tile([D, L_LOC], f32, tag="pa")
            nc.tensor.matmul(psum_k[:], W1T[0][:], p16[0][:],
                             start=True, stop=False)
            nc.tensor.matmul(psum_k[:], W1T[1][:], p16[1][:],
                             start=False, stop=True)
            k16 = wp.tile([D, L_LOC], f16)
            nc.scalar.activation(k16[:], psum_k[:], AF.Relu, bias=b1c)

            psum_pa2 = ps.tile([128, L_LOC], f32, tag="pb")
            nc.tensor.matmul(psum_pa2[:], WpaT2, k16[:], start=True, stop=True)
            pa2 = wp.tile([128, L_LOC], f16)
            nc.vector.tensor_scalar(pa2[:], psum_pa2[:], bpa2c, None, ALU.add)

            # ---------- omega_G from G table ----------
            # Gflat[64, 0:8]=top half of G2, [64, 8:16]=bottom half
            Gflat = wp.tile([64, B], f32)
            dmas(Gflat[:, 0:NB2], G2[0:64, :])
            dmas(Gflat[:, NB2:B], G2[64:128, :])
            omgG = wp.tile([128, B], f32)
            nc.gpsimd.memset(omgG[:], 0.0)
            t1g = wp.tile([64, B - 2], f32)
            nc.vector.tensor_tensor(t1g[:], Gflat[:, 2:B], Gflat[:, 0:B - 2],
                                    op=ALU.add)
            nc.vector.scalar_tensor_tensor(omgG[0:64, 1:B - 1],
                                           Gflat[:, 1:B - 1], -2.0, t1g[:],
                                           op0=ALU.mult, op1=ALU.add)
            nc.vector.tensor_tensor(omgG[0:64, 0:1], Gflat[:, 1:2],
                                    Gflat[:, 0:1], op=ALU.subtract)
            constG = wp.tile([64, 1], f32)
            nc.vector.scalar_tensor_tensor(constG[:], omgG[0:64, 0:1],
                                           float(-_t[0] / KH), Gflat[:, 0:1],
                                           op0=ALU.mult, op1=ALU.add)
            # omega pair layout for the folds: col j = [omg[15-j]; omg[7-j]]
            omgG2 = wp.tile([128, NB2], f32)
            dmas(omgG2[0:64, :], omgG[0:64, NB2:B][:, ::-1])
            dmas(omgG2[64:128, :], omgG[0:64, 0:NB2][:, ::-1])

            lhsG0 = wp.tile([128, D], f16)
            nc.vector.tensor_scalar(lhsG0[:], Itoph, omgG[:, 0:1], None,
                                    ALU.mult)
            lhsG = []
            for j in range(NB2):
                lg = wp.tile([128, D], f16, name=f"lhsG{j}")
                nc.vector.tensor_scalar(lg[:], I2h, omgG2[:, j:j + 1], None,
                                        ALU.mult)
                lhsG.append(lg)

            # ---------- F table build (from pa2) + S_p eval tiles ----------
            F2 = wp.tile([128, NB2], f32)
            TP = []
            for j in range(NB2):
                tp_ = wp.tile([128, L_LOC], f16, name=f"TP{j}", tag=f"TP{j}")
                if ACT_ROUTE[j]:
                    nc.scalar.activation(tp_[:], pa2[:], AF.Relu,
                                         bias=tcol[j],
                                         accum_out=F2[:, j:j + 1])
                else:
                    nc.vector.tensor_scalar(tp_[:], pa2[:], ntcol[j], tcol[j],
                                            ALU.max, ALU.add)
                    nc.vector.reduce_sum(F2[:, j:j + 1], tp_[:], axis=AX.X)
                TP.append(tp_)

            # ---------- S_p fold + p_gate + prot ----------
            psum_sp = ps.tile([D, L_LOC], f32, tag="pc0")
            nc.tensor.matmul(psum_sp[:], lhsG0[:], pa2[:],
                             start=True, stop=False)
            for j in range(NB2):
                nc.tensor.matmul(psum_sp[:], lhsG[j][:], TP[j][:],
                                 start=False, stop=(j == NB2 - 1))
            Sp16 = wp.tile([D, L_LOC], f16)
            nc.scalar.activation(Sp16[:], psum_sp[:], AF.Identity, bias=constG)

            psum_pg = ps.tile([D, L_LOC], f32, tag="pa")
            nc.tensor.matmul(psum_pg[:], WaT, Sp16[:], start=True, stop=True)
            pgate = wp.tile([D, L_LOC], f16)
            nc.scalar.activation(pgate[:], psum_pg[:], AF.Sigmoid,
                                 bias=bac, scale=1.0 / Q)
            g16 = wp.tile([D, L_LOC], f16)
            nc.vector.scalar_tensor_tensor(g16[:], pgate[:], 1.0, k16[:],
                                           op0=ALU.add, op1=ALU.mult)
            prot = wp.tile([D, 1], f32)
            nc.vector.reduce_max(prot[:], g16[:], axis=AX.X)
            protcols = wp.tile([D, NCORES], f32)
            nc.vector.tensor_scalar(protcols[:], mask8[:], prot[:], None,
                                    ALU.mult)

            # ---------- collective: AllReduce([64,24] f32) ----------
            cc_in = dp.tile([64, B + NCORES], f32)
            cc_out = dp.tile([64, B + NCORES], f32, addr_space="Shared")
            dmas(cc_in[:, 0:NB2], F2[0:64, :])
            dmas(cc_in[:, NB2:B], F2[64:128, :])
            dmas(cc_in[:, B:B + NCORES], protcols[:])
            nc.gpsimd.collective_compute(
                "AllReduce", ALU.add,
                replica_groups=[list(range(NCORES))],
                ins=[cc_in[:].opt()],
                outs=[cc_out[:].opt()],
            )
            post = wp.tile([64, B + NCORES], f32)
            dmas(post[:], cc_out[:])
            Fflat = post[:, 0:B]

            # ---------- omega_F + S_r fold + r_gate ----------
            protg = wp.tile([D, 1], f32)
            nc.vector.reduce_max(protg[:], post[:, B:B + NCORES], axis=AX.X)
            protg16 = wp.tile([D, 1], f16)
            nc.vector.tensor_scalar(protg16[:], protg[:], 1.0, None, ALU.mult)

            omgF = wp.tile([128, B], f32)
            nc.gpsimd.memset(omgF[:], 0.0)
            t1f = wp.tile([64, B - 2], f32)
            nc.vector.tensor_tensor(t1f[:], Fflat[:, 2:B], Fflat[:, 0:B - 2],
                                    op=ALU.add)
            nc.vector.scalar_tensor_tensor(omgF[0:64, 1:B - 1],
                                           Fflat[:, 1:B - 1], -2.0, t1f[:],
                                           op0=ALU.mult, op1=ALU.add)
            nc.vector.tensor_tensor(omgF[0:64, 0:1], Fflat[:, 1:2],
                                    Fflat[:, 0:1], op=ALU.subtract)
            constF = wp.tile([64, 1], f32)
            nc.vector.scalar_tensor_tensor(constF[:], omgF[0:64, 0:1],
                                           float(-_t[0] / KH), Fflat[:, 0:1],
                                           op0=ALU.mult, op1=ALU.add)
            omgF2 = wp.tile([128, NB2], f32)
            dmas(omgF2[0:64, :], omgF[0:64, NB2:B][:, ::-1])
            dmas(omgF2[64:128, :], omgF[0:64, 0:NB2][:, ::-1])

            lhsF0 = wp.tile([128, D], f16)
            nc.vector.tensor_scalar(lhsF0[:], Itoph, omgF[:, 0:1], None,
                                    ALU.mult)
            lhsF = []
            for j in range(NB2):
                lf = wp.tile([128, D], f16, name=f"lhsF{j}")
                nc.vector.tensor_scalar(lf[:], I2h, omgF2[:, j:j + 1], None,
                                        ALU.mult)
                lhsF.append(lf)

            psum_sr = ps.tile([D, Q], f32, tag="pb")
            nc.tensor.matmul(psum_sr[:], lhsF0[:], ra2[:],
                             start=True, stop=False)
            for j in range(NB2):
                nc.tensor.matmul(psum_sr[:], lhsF[j][:], TR[j][:],
                                 start=False, stop=(j == NB2 - 1))
            Sr16 = wp.tile([D, Q], f16)
            nc.scalar.activation(Sr16[:], psum_sr[:], AF.Identity, bias=constF)

            psum_rg = ps.tile([D, Q], f32, tag="pa")
            nc.tensor.matmul(psum_rg[:], WaT, Sr16[:], start=True, stop=True)
            rgate = wp.tile([D, Q], f16)
            nc.scalar.activation(rgate[:], psum_rg[:], AF.Sigmoid,
                                 bias=bac, scale=1.0 / L)
            rx16 = wp.tile([D, Q], f16)
            nc.vector.scalar_tensor_tensor(rx16[:], rgate[:], 1.0, r16[:],
                                           op0=ALU.add, op1=ALU.mult)

            # ---------- MLP head ----------
            h1 = []
            for m in range(2):
                psum_f = ps.tile([128, 1], f32, tag="pt")
                nc.tensor.matmul(psum_f[:],
                                 Wf1bT[:, m * 128:(m + 1) * 128],
                                 protg16[:], start=True, stop=True)
                foldb = wp.tile([128, 1], f32, name=f"foldb{m}")
                nc.scalar.activation(foldb[:], psum_f[:], AF.Identity,
                                     bias=bf1c[m])
                psum_h1 = ps.tile([128, Q], f32, tag=f"pc{m}")
                nc.tensor.matmul(psum_h1[:],
                                 Wf1aT[:, m * 128:(m + 1) * 128],
                                 rx16[:], start=True, stop=True)
                h1l = wp.tile([128, Q], f32, name=f"h1l{m}")
                nc.scalar.activation(h1l[:], psum_h1[:], AF.Identity,
                                     bias=foldb[:])
                h1m = wp.tile([128, Q], f16, name=f"h1_{m}")
                nc.vector.scalar_tensor_tensor(h1m[:], h1l[:], 0.01, h1l[:],
                                               op0=ALU.mult, op1=ALU.max)
                h1.append(h1m)

            psum_h2 = ps.tile([128, Q], f32, tag="pb")
            nc.tensor.matmul(psum_h2[:], Wf2T[0][:], h1[0][:],
                             start=True, stop=False)
            nc.tensor.matmul(psum_h2[:], Wf2T[1][:], h1[1][:],
                             start=False, stop=True)
            h2l = wp.tile([128, Q], f32)
            nc.scalar.activation(h2l[:], psum_h2[:], AF.Identity, bias=bf2c)
            h2 = wp.tile([128, Q], f16)
            nc.vector.scalar_tensor_tensor(h2[:], h2l[:], 0.01, h2l[:],
                                           op0=ALU.mult, op1=ALU.max)

            psum_o = ps.tile([1, Q], f32, tag="pa")
            nc.tensor.matmul(psum_o[:], Wf3T, h2[:], start=True, stop=True)
            out_sb = wp.tile([1, Q], f32)
            nc.scalar.activation(out_sb[:], psum_o[:], AF.Identity, bias=bf3c)
            dmas(out_d, out_sb[:])

    nc.compile()
    return nc


def _get_nc():
    key = ("v2", KR, ACT_ROUTE)
    if key not in _CACHE:
        _CACHE[key] = _build()
    return _CACHE[key]


def _prep_in_maps(inputs):
    f16 = np.float16
    f = lambda x: np.ascontiguousarray(np.asarray(x), dtype=np.float32)
    protein = f(inputs["protein"])[0]          # [L, 1024]
    reactions = f(inputs["reactions"])[0]      # [Q, 256]
    Wc, bc = f(inputs["Wc"]), f(inputs["bc"])
    W1, b1 = f(inputs["W1"]), f(inputs["b1"])
    W2, b2 = f(inputs["W2"]), f(inputs["b2"])
    Wa, ba = f(inputs["Wa"]), f(inputs["ba"])
    Wpa, bpa = f(inputs["Wpa"]), f(inputs["bpa"])
    Wra, bra = f(inputs["Wra"]), f(inputs["bra"])
    Wf1, bf1 = f(inputs["Wf1"]), f(inputs["bf1"])
    Wf2, bf2 = f(inputs["Wf2"]), f(inputs["bf2"])
    Wf3, bf3 = f(inputs["Wf3"]), f(inputs["bf3"])

    cst32 = np.zeros((128, 27), np.float32)
    cst32[0:64, 0] = b2
    cst32[:, 1] = np.tile(bra, 2)
    cst32[:, 2] = bc[0:128]
    cst32[:, 3] = bc[128:256]
    cst32[0:64, 4] = b1
    cst32[:, 5] = np.tile(bpa, 2)
    cst32[0:64, 6] = ba
    cst32[:, 7] = bf1[0:128]
    cst32[:, 8] = bf1[128:256]
    cst32[:, 9] = bf2
    cst32[0, 10] = bf3[0]
    for j in range(NB2):
        cst32[0:64, 11 + j] = _t[j]
        cst32[64:128, 11 + j] = _t[j + NB2]
        cst32[0:64, 19 + j] = -_t[j]
        cst32[64:128, 19 + j] = -_t[j + NB2]

    cstA = np.zeros((128, 1280), np.float16)
    cstA[:, 0:512] = reactions.T[0:128, :].astype(f16)
    cstA[:, 512:1024] = reactions.T[128:256, :].astype(f16)
    cstA[:, 1024:1088] = W2.T[0:128, :].astype(f16)
    cstA[:, 1088:1152] = W2.T[128:256, :].astype(f16)
    cstA[0:64, 1152:1280] = np.concatenate([Wra.T, Wra.T], 1).astype(f16)

    cstW = np.zeros((128, 2048), np.float16)
    for i in range(8):
        cstW[:, i * 256:(i + 1) * 256] = Wc.T[i * 128:(i + 1) * 128, :]

    cstB = np.zeros((128, 1217), np.float16)
    cstB[:, 0:64] = W1.T[0:128, :].astype(f16)
    cstB[:, 64:128] = W1.T[128:256, :].astype(f16)
    cstB[0:64, 128:256] = np.concatenate([Wpa.T, Wpa.T], 1).astype(f16)
    cstB[0:64, 256:320] = Wa.T.astype(f16)
    cstB[0:64, 320:576] = Wf1[:, 0:64].T.astype(f16)
    cstB[0:64, 576:832] = Wf1[:, 64:128].T.astype(f16)
    cstB[:, 832:960] = Wf2.T[0:128, :].astype(f16)
    cstB[:, 960:1088] = Wf2.T[128:256, :].astype(f16)
    cstB[:, 1088:1089] = Wf3.T.astype(f16)
    I2 = np.concatenate([np.eye(D), np.eye(D)], 0) / KH
    cstB[:, 1089:1153] = I2.astype(f16)
    Itop = np.concatenate([np.eye(D) / KH, np.zeros((D, D))], 0)
    cstB[:, 1153:1217] = Itop.astype(f16)

    common = {"cst32": cst32, "cstA": cstA, "cstW": cstW, "cstB": cstB}
    in_maps = []
    for d in range(NCORES):
        shard = protein[d * L_LOC:(d + 1) * L_LOC, :].T  # [1024, 512]
        protT = np.ascontiguousarray(
            shard.reshape(8, 128, L_LOC).transpose(1, 0, 2).reshape(
                128, 4096)).astype(f16)
        mask8 = np.zeros((64, NCORES), np.float32)
        mask8[:, d] = 1.0
        in_maps.append({**common, "protT": protT, "mask8": mask8})
    return in_maps


def run(inputs, trace=False, **kw):
    from concourse import bass_utils
    nc = _get_nc()
    in_maps = _prep_in_maps(inputs)
    res = bass_utils.run_bass_kernel_spmd(
        nc, in_maps, core_ids=list(range(NCORES)), trace=trace, **kw)
    return res


def kernel(**inputs):
    res = run(inputs)
    return np.asarray(res.results[0]["out"], np.float32).reshape(-1)
